# revision 8
# baseline (speedup 1.0000x reference)
"""AFT (Attention-Free Transformer) encoder block on 8 TRN2 NeuronCores.

Strategy
--------
Two SPMD launches:

Phase 1 (sequence-sharded): each core takes a T/8 slice of the sequence
axis for ALL batches, computes K = LN1(x) @ Wk for its slice and reduces
max over the batch axis locally -> M0 slice [T/8, D].  The host merely
concatenates the 8 slices (pure gather).

Phase 2 (batch-sharded): each core owns one batch element and computes the
whole block.  The batch-max M0 (replicated input) makes exp_K local.

Math notes:
 - exp_w's row-max stabilization cancels exactly in num/den (per-row
   factor), so we use exp(w) directly -> no row-max, no extra pass.
 - bk cancels between K and max_b(K) (max is shift-equivariant), so the
   K-projection bias is skipped in both phases (exact when bk == 0).

All matmuls run as float32r (= tf32: full-rate on the PE at moving dim
>= 256).  Matmul operand tiles are f32r-typed: compute-produced ones are
rounded by their producing ACT/DVE write; DMA'd weights are raw-bit
loads (PE reads tf32 precision either way).  Transposes stay plain fp32
(f32r transpose fails walrus codegen).  Layout plan per core (phase 2), all tiles [128, *] fp32:
  hT   = LN1(x)^T              4x[128,2048]  (PE transpose of h tiles)
  E    = exp(K - M0)          16x[128, 512]  (natural [t,d] rows)
  U    = E * (h@Wv + bv)      16x[128, 512]
  GT   = sigmoid(Q)^T          4x[128,2048]  ([d,t] via lhsT=Wq, rhs=hT)
  EW   = exp(w)^T streamed     [2048s, 256t] columns (PE transpose + ACT exp)
  numT/denT accumulate in PSUM via lhsT=U/E slices ([s,d] natural), rhs=EW
  Yt2T = GT * numT/denT        4x[128,2048]  ([d,t], reuses hT slots)
  attn = Yt2T^T @ Wo + bo + x  natural [t,d] (lhsT=Yt2T slices, rhs=Wo)
  LN2 -> h2 -> transpose -> h2T (reuses Yt2T slots);
  FFN: g1T[h,t] = gelu(W1^T h2T + b1), out += gelu(g1T^T @ W2 + b2).
"""

import sys

for _p in ("/opt/trn_rl_repo",):
    if _p not in sys.path:
        sys.path.insert(0, _p)

import numpy as np

import concourse.bass as bass
import concourse.bacc as bacc
import concourse.tile as tile
from concourse import mybir
from concourse import bass_utils
from concourse.masks import make_identity

B, T, D, H = 8, 2048, 512, 2048
EPS = 1e-5
NCORES = 8
P = 128
TS = T // NCORES          # seq rows per core in phase 1
NT = T // P               # 16 row tiles of the full sequence
ND = D // P               # 4 d-chunks
NH = H // P               # 16 h-chunks
TCH = 256                 # einsum t-chunk (moving dim)
NTC = T // TCH            # 8 einsum t-chunks
F32 = mybir.dt.float32
F32R = mybir.dt.float32r
AF = mybir.ActivationFunctionType
ALU = mybir.AluOpType
PSUM = bass.MemorySpace.PSUM

TRACE = False             # test harness sets True to capture NTFF profiles
LAST_RESULTS = []         # BassKernelResults per phase from the last kernel()


def _r(ap):
    """View an fp32 AP as float32r for full-rate PE matmul."""
    return ap.bitcast(F32R)


def _ln_tile(nc, pool, x_tile, g_bc, b_bc, eps_tile):
    """LayerNorm one [P, D] tile; returns normalized tile from `pool`."""
    stats = pool.tile([P, 6], F32, tag="ln_stats")
    mv = pool.tile([P, 2], F32, tag="ln_mv")
    nc.vector.bn_stats(out=stats, in_=x_tile)
    nc.vector.bn_aggr(out=mv, in_=stats)
    rstd = pool.tile([P, 1], F32, tag="ln_rstd")
    nc.scalar.activation(out=rstd, in_=mv[:, 1:2], func=AF.Sqrt, bias=eps_tile)
    nc.vector.reciprocal(out=rstd, in_=rstd)
    h_tile = pool.tile([P, D], F32, tag="ln_h")
    nc.vector.tensor_scalar(
        out=h_tile, in0=x_tile,
        scalar1=mv[:, 0:1], scalar2=rstd,
        op0=ALU.subtract, op1=ALU.mult)
    nc.vector.tensor_mul(out=h_tile, in0=h_tile, in1=g_bc)
    nc.vector.tensor_add(out=h_tile, in0=h_tile, in1=b_bc)
    return h_tile


def _transpose_tile(nc, psum_pool, dst, dst_col, src, identity):
    """dst[:, dst_col:dst_col+P] = src[:, :P].T via PE transpose."""
    pt = psum_pool.tile([P, P], F32, tag="tpsum")
    nc.tensor.transpose(pt, src, identity)
    nc.vector.tensor_copy(out=dst[:, dst_col:dst_col + P], in_=pt)


def _load_rows(nc, pool, dram_ap, n_tiles, tag, width, dtype=F32):
    """Load [P, width] row tiles of a DRAM matrix into a list of tiles.

    dtype=F32R does a raw-bits DMA into an f32r-typed tile (the PE reads
    tf32 precision either way; the verifier accepts DMA producers).
    """
    out = []
    for j in range(n_tiles):
        t = pool.tile([P, width], dtype, tag=f"{tag}{j}")
        src = dram_ap[j * P:(j + 1) * P, :]
        if dtype is F32R:
            src = src.bitcast(F32R)
        nc.sync.dma_start(out=t, in_=src)
        out.append(t)
    return out


def _bc(nc, pool, dram_ap, tag):
    """Broadcast a [D] vector across 128 partitions."""
    t = pool.tile([P, D], F32, tag=tag)
    nc.gpsimd.dma_start(out=t, in_=dram_ap.partition_broadcast(P))
    return t


def _part_bias(nc, pool, dram_ap, n, tag):
    """Load a [n*P] vector as per-partition bias columns [P, n]."""
    t = pool.tile([P, n], F32, tag=tag)
    for k in range(n):
        nc.sync.dma_start(
            out=t[:, k:k + 1],
            in_=dram_ap[k * P:(k + 1) * P].rearrange("(p o) -> p o", o=1))
    return t


def _build_phase1():
    """Per core: rows = [B, TS] b-major flattened; out M0 = max_b (LN1(x)@Wk)."""
    nc = bacc.Bacc(trn_type="TRN2", target_bir_lowering=False, debug=False,
                   num_devices=NCORES)
    xs = nc.dram_tensor("xs", [B * TS, D], F32, kind="ExternalInput").ap()
    g1 = nc.dram_tensor("ln1_g", [D], F32, kind="ExternalInput").ap()
    b1 = nc.dram_tensor("ln1_b", [D], F32, kind="ExternalInput").ap()
    wk = nc.dram_tensor("Wk", [D, D], F32, kind="ExternalInput").ap()
    m0 = nc.dram_tensor("M0", [TS, D], F32, kind="ExternalOutput").ap()

    n_tiles = B * TS // P          # 16
    tiles_per_b = TS // P          # 2

    with tile.TileContext(nc) as tc:
        pools = []

        def alloc(**kw):
            p = tc.alloc_tile_pool(**kw)
            pools.append(p)
            return p

        pc = alloc(name="consts", bufs=1)
        pwk = alloc(name="wk", bufs=1)
        pk = alloc(name="ks", bufs=1)
        ps = alloc(name="stream", bufs=3)
        ppt = alloc(name="psum_t", bufs=2, space=PSUM)
        ppm = alloc(name="psum_mm", bufs=2, space=PSUM)

        identity = pc.tile([P, P], F32)
        make_identity(nc, identity)
        eps_tile = pc.tile([P, 1], F32)
        nc.vector.memset(eps_tile, EPS)
        g_bc = _bc(nc, pc, g1, "g_bc")
        b_bc = _bc(nc, pc, b1, "b_bc")
        wk_sb = _load_rows(nc, pwk, wk, ND, "wk", D, dtype=F32R)

        k_sb = []
        for j in range(n_tiles):
            x_tile = ps.tile([P, D], F32, tag="x")
            nc.sync.dma_start(out=x_tile, in_=xs[j * P:(j + 1) * P, :])
            h_tile = _ln_tile(nc, ps, x_tile, g_bc, b_bc, eps_tile)
            hT = ps.tile([P, P * ND], F32R, tag="hT")
            for dj in range(ND):
                _transpose_tile(nc, ppt, hT, dj * P,
                                h_tile[:, dj * P:(dj + 1) * P], identity)
            pk_t = ppm.tile([P, D], F32, tag="kpsum")
            for dj in range(ND):
                nc.tensor.matmul(
                    pk_t, hT[:, dj * P:(dj + 1) * P], wk_sb[dj],
                    start=(dj == 0), stop=(dj == ND - 1))
            kt = pk.tile([P, D], F32, tag=f"k{j}")
            nc.vector.tensor_copy(out=kt, in_=pk_t)
            k_sb.append(kt)

        for half in range(tiles_per_b):
            acc = ps.tile([P, D], F32, tag="macc")
            nc.vector.tensor_tensor(
                out=acc, in0=k_sb[half], in1=k_sb[tiles_per_b + half],
                op=ALU.max)
            for b in range(2, B):
                nc.vector.tensor_tensor(
                    out=acc, in0=acc, in1=k_sb[b * tiles_per_b + half],
                    op=ALU.max)
            nc.sync.dma_start(out=m0[half * P:(half + 1) * P, :], in_=acc)

        for p in reversed(pools):
            p.release()

    nc.compile()
    return nc


def _build_phase2():
    nc = bacc.Bacc(trn_type="TRN2", target_bir_lowering=False, debug=False,
                   num_devices=NCORES)
    ap = {}
    ap["x"] = nc.dram_tensor("x", [T, D], F32, kind="ExternalInput").ap()
    ap["M0"] = nc.dram_tensor("M0", [T, D], F32, kind="ExternalInput").ap()
    ap["w"] = nc.dram_tensor("w", [T, T], F32, kind="ExternalInput").ap()
    for n, shp in [("ln1_g", [D]), ("ln1_b", [D]), ("Wk", [D, D]),
                   ("Wv", [D, D]), ("bv", [D]), ("Wq", [D, D]), ("bq", [D]),
                   ("Wo", [D, D]), ("bo", [D]), ("ln2_g", [D]), ("ln2_b", [D]),
                   ("W1", [D, H]), ("b1", [H]), ("W2", [H, D]), ("b2", [D])]:
        ap[n] = nc.dram_tensor(n, shp, F32, kind="ExternalInput").ap()
    out_d = nc.dram_tensor("out", [T, D], F32, kind="ExternalOutput").ap()

    with tile.TileContext(nc) as tc:
        # SBUF pool stack (LIFO release):
        #   pc | ptm (hT -> Yt2T -> h2T) | pe | pu | pg | pw | ps_a
        #   ... A, A2 ... pop ps_a, pw
        #   push pew, pbs ... B ... pop pbs, pew, pg, pu, pe
        #   push pout, pwo, ps_c ... C ... pop ps_c, pwo
        #   push pfw, pg1, pds ... D ... pop all
        pc = tc.alloc_tile_pool(name="consts", bufs=1)
        ptm = tc.alloc_tile_pool(name="tmat", bufs=1)
        pe = tc.alloc_tile_pool(name="rows_e", bufs=1)
        pu = tc.alloc_tile_pool(name="rows_u", bufs=1)
        pg = tc.alloc_tile_pool(name="gate", bufs=1)
        pw = tc.alloc_tile_pool(name="wproj", bufs=1)
        ps_a = tc.alloc_tile_pool(name="stream_a", bufs=2)
        # PSUM stack: ppt | ppm | (ppnd in B) | (ppg, ppa in D)
        ppt = tc.alloc_tile_pool(name="psum_t", bufs=2, space=PSUM)
        ppm = tc.alloc_tile_pool(name="psum_mm", bufs=2, space=PSUM)

        identity = pc.tile([P, P], F32)
        make_identity(nc, identity)
        eps_tile = pc.tile([P, 1], F32)
        nc.vector.memset(eps_tile, EPS)

        # ---------------- Stage A: LN1, hT, E, U -----------------------
        g1_bc = _bc(nc, pw, ap["ln1_g"], "g1_bc")
        b1g_bc = _bc(nc, pw, ap["ln1_b"], "b1g_bc")
        bv_bc = _bc(nc, pw, ap["bv"], "bv_bc")
        wk_sb = _load_rows(nc, pw, ap["Wk"], ND, "wk", D, dtype=F32R)
        wv_sb = _load_rows(nc, pw, ap["Wv"], ND, "wv", D, dtype=F32R)
        wq_sb = _load_rows(nc, pw, ap["Wq"], ND, "wq", D, dtype=F32R)

        hT = [ptm.tile([P, T], F32R, tag=f"tm{dj}", name=f"hT{dj}")
              for dj in range(ND)]
        e_sb, u_sb = [], []
        for j in range(NT):
            x_tile = ps_a.tile([P, D], F32, tag="x")
            nc.sync.dma_start(out=x_tile, in_=ap["x"][j * P:(j + 1) * P, :])
            h_tile = _ln_tile(nc, ps_a, x_tile, g1_bc, b1g_bc, eps_tile)
            for dj in range(ND):
                _transpose_tile(nc, ppt, hT[dj], j * P,
                                h_tile[:, dj * P:(dj + 1) * P], identity)
            # K (no bias; cancels with M0)
            pk_t = ppm.tile([P, D], F32, tag="kv_psum")
            for dj in range(ND):
                nc.tensor.matmul(
                    pk_t, hT[dj][:, j * P:(j + 1) * P], wk_sb[dj],
                    start=(dj == 0), stop=(dj == ND - 1))
            m_tile = ps_a.tile([P, D], F32, tag="m0")
            nc.sync.dma_start(out=m_tile, in_=ap["M0"][j * P:(j + 1) * P, :])
            ein = ps_a.tile([P, D], F32, tag="ein")
            nc.vector.tensor_sub(out=ein, in0=pk_t, in1=m_tile)
            et = pe.tile([P, D], F32R, tag=f"e{j}")
            nc.scalar.activation(out=et, in_=ein, func=AF.Exp)
            e_sb.append(et)
            # V then U = E * (V + bv)
            pv_t = ppm.tile([P, D], F32, tag="kv_psum")
            for dj in range(ND):
                nc.tensor.matmul(
                    pv_t, hT[dj][:, j * P:(j + 1) * P], wv_sb[dj],
                    start=(dj == 0), stop=(dj == ND - 1))
            v1 = ps_a.tile([P, D], F32, tag="v1")
            nc.vector.tensor_add(out=v1, in0=pv_t, in1=bv_bc)
            ut = pu.tile([P, D], F32R, tag=f"u{j}")
            nc.vector.tensor_mul(out=ut, in0=et.bitcast(F32), in1=v1)
            u_sb.append(ut)

        # ---------- Stage A2: GT = sigmoid(Q)^T ------------------------
        bq_sb = _part_bias(nc, pg, ap["bq"], ND, "bq_sb")
        gt = [pg.tile([P, T], F32, tag=f"gt{dk}", name=f"gt{dk}")
              for dk in range(ND)]
        for dk in range(ND):
            for ts4 in range(T // 512):
                pq_t = ppm.tile([P, 512], F32, tag="kv_psum")
                for dj in range(ND):
                    nc.tensor.matmul(
                        pq_t,
                        wq_sb[dj][:, dk * P:(dk + 1) * P],
                        hT[dj][:, ts4 * 512:(ts4 + 1) * 512],
                        start=(dj == 0), stop=(dj == ND - 1))
                nc.scalar.activation(
                    out=gt[dk][:, ts4 * 512:(ts4 + 1) * 512], in_=pq_t,
                    func=AF.Sigmoid, bias=bq_sb[:, dk:dk + 1])
        ps_a.release()
        pw.release()

        # ---------- Stage B: einsum + gate -----------------------------
        pew = tc.alloc_tile_pool(name="ew", bufs=2)
        pbs = tc.alloc_tile_pool(name="bstream", bufs=2)
        ppnd = tc.alloc_tile_pool(name="psum_nd", bufs=2, space=PSUM)

        yt2 = [ptm.tile([P, T], F32R, tag=f"tm{dj}", name=f"yt2_{dj}")
               for dj in range(ND)]
        for tk in range(NTC):
            # build EW[:, tk-chunk]: [2048 s, 256 t] = exp(w[tchunk, :])^T
            ew = [pew.tile([P, TCH], F32R, tag=f"ew{si}", name=f"ew{si}")
                  for si in range(NT)]
            for u in range(TCH // P):
                wrow = pbs.tile([P, T], F32, tag="wrow")
                nc.sync.dma_start(
                    out=wrow,
                    in_=ap["w"][tk * TCH + u * P:tk * TCH + (u + 1) * P, :])
                for si in range(NT):
                    pt = ppt.tile([P, P], F32, tag="tpsum")
                    nc.tensor.transpose(
                        pt, wrow[:, si * P:(si + 1) * P], identity)
                    nc.scalar.activation(
                        out=ew[si][:, u * P:(u + 1) * P], in_=pt, func=AF.Exp)
            for dj in range(ND):
                pn = ppnd.tile([P, TCH], F32, tag="num")
                pd = ppnd.tile([P, TCH], F32, tag="den")
                for si in range(NT):
                    nc.tensor.matmul(
                        pn, u_sb[si][:, dj * P:(dj + 1) * P], ew[si],
                        start=(si == 0), stop=(si == NT - 1))
                for si in range(NT):
                    nc.tensor.matmul(
                        pd, e_sb[si][:, dj * P:(dj + 1) * P], ew[si],
                        start=(si == 0), stop=(si == NT - 1))
                rec = pbs.tile([P, TCH], F32, tag="rec")
                nc.vector.reciprocal(out=rec, in_=pd)
                ytt = pbs.tile([P, TCH], F32, tag="ytt")
                nc.vector.tensor_mul(out=ytt, in0=pn, in1=rec)
                nc.vector.tensor_mul(
                    out=yt2[dj][:, tk * TCH:(tk + 1) * TCH],
                    in0=ytt, in1=gt[dj][:, tk * TCH:(tk + 1) * TCH])
        ppnd.release()
        pbs.release()
        pew.release()
        pg.release()
        pu.release()
        pe.release()

        # ------ Stage C: attn-out + residual + LN2 + h2T ---------------
        pout = tc.alloc_tile_pool(name="rows_out", bufs=1)
        pwo = tc.alloc_tile_pool(name="wo", bufs=1)
        ps_c = tc.alloc_tile_pool(name="stream_c", bufs=2)

        wo_sb = _load_rows(nc, pwo, ap["Wo"], ND, "wo", D, dtype=F32R)
        bo_bc = _bc(nc, pwo, ap["bo"], "bo_bc")
        g2_bc = _bc(nc, pwo, ap["ln2_g"], "g2_bc")
        b2g_bc = _bc(nc, pwo, ap["ln2_b"], "b2g_bc")

        out_sb = []
        attn_sb = []
        for j in range(NT):
            pa_t = ppm.tile([P, D], F32, tag="kv_psum")
            for dj in range(ND):
                nc.tensor.matmul(
                    pa_t, yt2[dj][:, j * P:(j + 1) * P], wo_sb[dj],
                    start=(dj == 0), stop=(dj == ND - 1))
            x_tile = ps_c.tile([P, D], F32, tag="x")
            nc.sync.dma_start(out=x_tile, in_=ap["x"][j * P:(j + 1) * P, :])
            ot = pout.tile([P, D], F32, tag=f"o{j}")
            nc.vector.tensor_add(out=ot, in0=pa_t, in1=bo_bc)
            nc.vector.tensor_add(out=ot, in0=ot, in1=x_tile)
            out_sb.append(ot)

        h2T = [ptm.tile([P, T], F32R, tag=f"tm{dj}", name=f"h2T{dj}")
               for dj in range(ND)]
        for j in range(NT):
            h2_tile = _ln_tile(nc, ps_c, out_sb[j], g2_bc, b2g_bc, eps_tile)
            for dj in range(ND):
                _transpose_tile(nc, ppt, h2T[dj], j * P,
                                h2_tile[:, dj * P:(dj + 1) * P], identity)
        ps_c.release()
        pwo.release()

        # ---------------- Stage D: FFN ---------------------------------
        pfw = tc.alloc_tile_pool(name="ffnw", bufs=1)
        pg1 = tc.alloc_tile_pool(name="g1", bufs=2)
        pds = tc.alloc_tile_pool(name="dstream", bufs=2)
        ppg = tc.alloc_tile_pool(name="psum_g", bufs=2, space=PSUM)
        ppa = tc.alloc_tile_pool(name="psum_a2", bufs=2, space=PSUM)

        w1_sb = _load_rows(nc, pfw, ap["W1"], ND, "w1_", H, dtype=F32R)
        w2_sb = _load_rows(nc, pfw, ap["W2"], NH, "w2_", D, dtype=F32R)
        b1_sb = _part_bias(nc, pfw, ap["b1"], NH, "b1_sb")
        b2_bc = _bc(nc, pfw, ap["b2"], "b2_bc")

        for ts2 in range(T // TCH):      # 8 strips of 256
            g1t = [pg1.tile([P, TCH], F32R, tag=f"g1_{hk}", name=f"g1t{hk}")
                   for hk in range(NH)]
            for hk in range(NH):
                pg_t = ppg.tile([P, TCH], F32, tag="g1psum")
                for dj in range(ND):
                    nc.tensor.matmul(
                        pg_t,
                        w1_sb[dj][:, hk * P:(hk + 1) * P],
                        h2T[dj][:, ts2 * TCH:(ts2 + 1) * TCH],
                        start=(dj == 0), stop=(dj == ND - 1))
                nc.scalar.activation(out=g1t[hk], in_=pg_t, func=AF.Gelu,
                                     bias=b1_sb[:, hk:hk + 1])
            for v in range(TCH // P):    # 2 row-tiles per strip
                j = ts2 * (TCH // P) + v
                pa2 = ppa.tile([P, D], F32, tag="a2psum")
                for hk in range(NH):
                    nc.tensor.matmul(
                        pa2, g1t[hk][:, v * P:(v + 1) * P], w2_sb[hk],
                        start=(hk == 0), stop=(hk == NH - 1))
                t1 = pds.tile([P, D], F32, tag="t1")
                nc.vector.tensor_add(out=t1, in0=pa2, in1=b2_bc)
                t2 = pds.tile([P, D], F32, tag="t2")
                nc.scalar.activation(out=t2, in_=t1, func=AF.Gelu)
                fin = pds.tile([P, D], F32, tag="fin")
                nc.vector.tensor_add(out=fin, in0=t2, in1=out_sb[j])
                nc.sync.dma_start(out=out_d[j * P:(j + 1) * P, :], in_=fin)

        for p in (ppa, ppg, pds, pg1, pfw, pout, ptm, pc, ppm, ppt):
            p.release()

    nc.compile()
    return nc


_CACHE = {}


def _get_programs():
    if "p1" not in _CACHE:
        _CACHE["p1"] = _build_phase1()
        _CACHE["p2"] = _build_phase2()
    return _CACHE["p1"], _CACHE["p2"]


def kernel(**inputs):
    np32 = {k: np.ascontiguousarray(np.asarray(v, dtype=np.float32))
            for k, v in inputs.items()}
    x = np32["x"]                      # [B, T, D]
    p1, p2 = _get_programs()
    LAST_RESULTS.clear()

    # Phase 1: seq-sharded batch-max of K
    in_maps1 = []
    for c in range(NCORES):
        xs = np.ascontiguousarray(
            x[:, c * TS:(c + 1) * TS, :].reshape(B * TS, D))
        in_maps1.append({
            "xs": xs,
            "ln1_g": np32["ln1_g"], "ln1_b": np32["ln1_b"],
            "Wk": np32["Wk"],
        })
    res1 = bass_utils.run_bass_kernel_spmd(p1, in_maps1,
                                           core_ids=list(range(NCORES)),
                                           trace=TRACE)
    LAST_RESULTS.append(res1)
    m0 = np.concatenate([res1.results[c]["M0"] for c in range(NCORES)], axis=0)

    # Phase 2: batch-sharded full block
    names = ["ln1_g", "ln1_b", "Wk", "Wv", "bv", "Wq", "bq", "Wo", "bo",
             "ln2_g", "ln2_b", "W1", "b1", "W2", "b2", "w"]
    shared = {n: np32[n] for n in names}
    in_maps2 = []
    for b in range(NCORES):
        m = {"x": np.ascontiguousarray(x[b]), "M0": m0}
        m.update(shared)
        in_maps2.append(m)
    res2 = bass_utils.run_bass_kernel_spmd(p2, in_maps2,
                                           core_ids=list(range(NCORES)),
                                           trace=TRACE)
    LAST_RESULTS.append(res2)
    out = np.stack([res2.results[b]["out"] for b in range(NCORES)], axis=0)
    return out
